# revision 1
# baseline (speedup 1.0000x reference)
"""Distributed attention-energy softmax kernel for 8 trn2 NeuronCores.

Math: reference computes
    energies = (enc @ W.T + b) @ h        # [S]
    attn     = softmax(energies)          # [1,1,S]
Algebraic rewrite: (enc @ W.T) @ h = enc @ (h^T W), and (b @ h) is a
constant added to every energy, which softmax is invariant to. So:
    v        = h^T W                      # [H]
    energies = enc @ v                    # [S]  (up to a constant shift)
    attn     = softmax(energies)

Sharding (8 cores):
  - enc [S=8192, H=2048] sharded along S: 1024 rows/core.
  - W sharded by COLUMNS: core c holds W[:, c*256:(c+1)*256] and computes
    its v slice v[c*256:(c+1)*256] = h @ W_shard on the PE (h replicated),
    then an AllGather of the [256] slices yields full v everywhere.
  - v broadcast to 128 partitions with ONE stride-0 DMA read of the
    AllGather result (no POOL partition_broadcast, no PE round trip).
  - energies: DVE multiply + ACT free-dim accumulate per [128, 2048]
    row-tile (big tiles amortize per-op engine overhead).
  - Global softmax: per-PARTITION max for the local exp, POOL max-reduce
    in parallel with the exp, PE ones-matmul for the cross-partition sum,
    one AllGather of the 8 (m_i, s_i) pairs, local rescale.
  - Engine-queue budget: DMA transfers occupy the issuing queue, so W and
    enc go on SP/ACT only; POOL stays free for the two collectives.

Layouts:
  - h input per core: [128, 16] with h_in[p, t] = h[t*128 + p] (replicated)
  - w input per core: [2048, 256] column slice of W
  - out per core: [128, 8] with out[p, t] = attn[core*1024 + t*128 + p]
"""

import numpy as np

H = 2048
S = 8192
N_CORES = 8
S_SHARD = S // N_CORES          # 1024
V_SHARD = H // N_CORES          # 256 v elements per core
N_TILES = S_SHARD // 128        # 8 row-tiles per core
N_CHUNKS = 8                    # enc DMA chunks (1 row-tile each)
KT = H // 128                   # 16 k-tiles for the v matvec


def emit(tc, out_ap, enc_ap, w_ap, h_ap, local=False, gate=None):
    """Emit the per-core kernel IR into TileContext tc.

    out_ap: [128, 8] f32; enc_ap: [1024, 2048]; w_ap: [2048, 256];
    h_ap: [128, 16]. local=True replaces collectives with plain DMA
    copies (single-core timeline simulation only). gate: optional [128,1]
    tile AP from a previous emit; serializes this iteration behind it
    (benchmarking N-in-one-NEFF loops). Returns a [128,1] gate tile.
    """
    import concourse.bass_isa as bass_isa
    import concourse.mybir as mybir

    nc = tc.nc
    f32 = mybir.dt.float32
    rg = [list(range(N_CORES))]
    Exp = mybir.ActivationFunctionType.Exp
    X = mybir.AxisListType.X
    mult = mybir.AluOpType.mult

    with (
        tc.tile_pool(name="const", bufs=1) as const,
        tc.tile_pool(name="enc_p", bufs=N_CHUNKS) as enc_p,
        tc.tile_pool(name="scratch", bufs=5) as scratch,
        tc.tile_pool(name="psum", bufs=1, space="PSUM") as psum,
        tc.tile_pool(name="dram", bufs=1, space="DRAM") as dram,
    ):
        ones_col = const.tile([128, 1], f32)
        nc.vector.memset(ones_col[:], 1.0)

        # ---- input DMAs; W first so the PE matvec (which gates the v
        # AllGather) is paced only by W arrival ----
        h_sb = const.tile([128, KT], f32)
        nc.scalar.dma_start(h_sb[:], h_ap)
        w_sb = const.tile([128, KT, V_SHARD], f32)
        w_re = w_ap.rearrange("(t p) k -> p t k", p=128)
        w_eng = [nc.sync, nc.scalar, nc.sync, nc.scalar]
        for i in range(4):
            w_eng[i].dma_start(
                w_sb[:, 4 * i : 4 * i + 4, :], w_re[:, 4 * i : 4 * i + 4, :]
            )
        if gate is not None:
            tok01 = const.tile([128, 1], f32)
            nc.vector.tensor_scalar(
                out=tok01[:], in0=gate, scalar1=0.0, scalar2=1.0,
                op0=mybir.AluOpType.mult, op1=mybir.AluOpType.add,
            )
            h_eff = const.tile([128, KT], f32)
            nc.vector.tensor_scalar_mul(h_eff[:], h_sb[:], tok01[:])
            h_sb = h_eff

        rows_per_chunk = S_SHARD // N_CHUNKS            # 256
        u_per_chunk = rows_per_chunk // 128             # 2
        enc_eng = [nc.sync, nc.scalar] * 4
        enc_tiles = []
        for t in range(N_CHUNKS):
            enc_c = enc_p.tile([128, u_per_chunk, H], f32, tag="enc_c")
            src = enc_ap[
                t * rows_per_chunk : (t + 1) * rows_per_chunk, :
            ].rearrange("(u p) h -> p u h", p=128)
            enc_eng[t].dma_start(enc_c[:], src)
            enc_tiles.append(enc_c)

        # ---- v slice: v[c*256:(c+1)*256] = h @ W[:, shard] on the PE ----
        vps = psum.tile([1, V_SHARD], f32)
        for t in range(KT):
            nc.tensor.matmul(
                vps[:],
                lhsT=h_sb[:, t : t + 1],
                rhs=w_sb[:, t, :],
                start=(t == 0),
                stop=(t == KT - 1),
            )
        v_row = const.tile([1, V_SHARD], f32)
        nc.vector.tensor_copy(v_row[:], vps[:])

        v_in_d = dram.tile([1, V_SHARD], f32)
        v_out_d = dram.tile([1, H], f32)
        nc.gpsimd.dma_start(v_in_d[:], v_row[:])
        if local:
            nc.gpsimd.dma_start(v_out_d[0:1, 0:V_SHARD], v_in_d[:])
        else:
            nc.gpsimd.collective_compute(
                "AllGather",
                mybir.AluOpType.bypass,
                replica_groups=rg,
                ins=[v_in_d.opt()],
                outs=[v_out_d.opt()],
            )
        # stride-0 DMAs replicate v across all 128 partitions in two
        # H-halves on separate queues, so the first multiply can start on
        # half A while half B is still streaming
        v_bc = const.tile([128, H], f32)
        nc.gpsimd.dma_start(
            v_bc[:, 0 : H // 2],
            v_out_d[0:1, 0 : H // 2].broadcast_to([128, H // 2]),
        )
        nc.sync.dma_start(
            v_bc[:, H // 2 : H],
            v_out_d[0:1, H // 2 : H].broadcast_to([128, H // 2]),
        )



        # ---- energies: multiplies split DVE/POOL, reduces split ACT/DVE.
        # Balanced for real-HW rates (DVE tensor_tensor ~2.2us, POOL ~4.4us,
        # ACT reduce ~2.1us per [128,2048] tile): DVE 5 mults + 1 reduce,
        # POOL 3 mults, ACT 7 reduces, each ~13-15us of queue time. ----
        e_sb = const.tile([128, N_TILES], f32)
        for t in range(N_CHUNKS):
            for u in range(u_per_chunk):
                idx = t * u_per_chunk + u
                mul_eng = nc.vector if idx < 5 else nc.gpsimd
                prod = scratch.tile([128, H], f32, tag="prod")
                if idx == 0:
                    # first tile: two half-multiplies on DVE and POOL in
                    # PARALLEL, each gated only on its own v_bc half
                    # (products land in one tile; the single ACT
                    # accumulation is unchanged)
                    for hh, eng in ((0, nc.vector), (1, nc.gpsimd)):
                        sl = slice(hh * (H // 2), (hh + 1) * (H // 2))
                        eng.tensor_tensor(
                            out=prod[:, sl],
                            in0=enc_tiles[t][:, u, sl],
                            in1=v_bc[:, sl],
                            op=mult,
                        )
                else:
                    mul_eng.tensor_tensor(
                        out=prod[:], in0=enc_tiles[t][:, u, :], in1=v_bc[:], op=mult
                    )
                if idx == N_TILES - 1:
                    nc.vector.tensor_reduce(
                        e_sb[:, idx : idx + 1], prod[:], axis=X,
                        op=mybir.AluOpType.add,
                    )
                else:
                    act_scr = scratch.tile([128, H], f32, tag="act_scr")
                    nc.scalar.activation(
                        act_scr[:],
                        prod[:],
                        mybir.ActivationFunctionType.Copy,
                        bias=0.0,
                        scale=1.0,
                        accum_out=e_sb[:, idx : idx + 1],
                    )

        # ---- local softmax stats (per-partition max keeps POOL off the
        #      exp critical path) ----
        nm_row = const.tile([128, 1], f32)  # -max_t e[p, t]
        nc.vector.tensor_reduce(
            nm_row[:], e_sb[:], axis=X, op=mybir.AluOpType.max, negate=True
        )
        m_row2 = const.tile([128, 1], f32)  # +max, reduced in parallel
        nc.vector.tensor_reduce(
            m_row2[:], e_sb[:], axis=X, op=mybir.AluOpType.max
        )
        m_loc = const.tile([128, 1], f32)  # core max, all partitions
        nc.gpsimd.partition_all_reduce(
            m_loc[:], m_row2[:], channels=128, reduce_op=bass_isa.ReduceOp.max
        )
        nm_loc = const.tile([128, 1], f32)
        nc.vector.tensor_scalar_mul(nm_loc[:], m_loc[:], -1.0)
        p_sb = const.tile([128, N_TILES], f32)  # exp(e - m_p) per partition
        s_row = const.tile([128, 1], f32)
        nc.scalar.activation(
            p_sb[:], e_sb[:], Exp, bias=nm_row[:], scale=1.0, accum_out=s_row[:]
        )
        # s_core = sum_p s_row[p] * exp(m_p - m_core)
        d_row = const.tile([128, 1], f32)
        nc.scalar.activation(d_row[:], nm_row[:], Exp, bias=nm_loc[:], scale=-1.0)
        t2 = const.tile([128, 1], f32)
        nc.vector.tensor_tensor(out=t2[:], in0=s_row[:], in1=d_row[:], op=mult)
        s_core_ps = psum.tile([1, 1], f32)
        nc.tensor.matmul(
            s_core_ps[:], lhsT=t2[:], rhs=ones_col[:], start=True, stop=True
        )

        # ---- AllGather the (m_i, s_i) pairs ----
        st_sb = const.tile([1, 2], f32)
        nc.vector.tensor_copy(st_sb[0:1, 0:1], m_loc[0:1, :])
        nc.vector.tensor_copy(st_sb[0:1, 1:2], s_core_ps[0:1, :])
        st_in_d = dram.tile([1, 2], f32)
        st_out_d = dram.tile([1, 2 * N_CORES], f32)
        nc.gpsimd.dma_start(st_in_d[:], st_sb[:])
        if local:
            nc.gpsimd.dma_start(st_out_d[0:1, 0:2], st_in_d[:])
        else:
            nc.gpsimd.collective_compute(
                "AllGather",
                mybir.AluOpType.bypass,
                replica_groups=rg,
                ins=[st_in_d.opt()],
                outs=[st_out_d.opt()],
            )
        # stride-0 DMA broadcast of the 16 gathered stats to all partitions
        allst = const.tile([128, 2 * N_CORES], f32)
        nc.sync.dma_start(
            allst[:], st_out_d[0:1, :].broadcast_to([128, 2 * N_CORES])
        )

        # ---- combine: c_p = exp(m_p - gmax) / gsum ----
        m_vec = allst[:, 0 : 2 * N_CORES : 2]
        s_vec = allst[:, 1 : 2 * N_CORES : 2]
        red = const.tile([128, 1], f32)  # -gmax
        nc.vector.tensor_reduce(
            red[:], m_vec, axis=X, op=mybir.AluOpType.max, negate=True
        )
        t_vec = const.tile([128, N_CORES], f32)
        nc.scalar.activation(t_vec[:], m_vec, Exp, bias=red[:], scale=1.0)
        tmp_vec = const.tile([128, N_CORES], f32)
        nc.vector.tensor_tensor(out=tmp_vec[:], in0=t_vec[:], in1=s_vec, op=mult)
        gsum = const.tile([128, 1], f32)
        nc.vector.tensor_reduce(gsum[:], tmp_vec[:], axis=X, op=mybir.AluOpType.add)
        ginv = const.tile([128, 1], f32)
        nc.vector.reciprocal(ginv[:], gsum[:])
        cexp = const.tile([128, 1], f32)
        nc.scalar.activation(cexp[:], nm_row[:], Exp, bias=red[:], scale=-1.0)

        # ---- finalize: attn_shard = exp(e - m_p) * exp(m_p - gmax) / gsum,
        # fused as one tensor_scalar with two per-partition scalars ----
        o_sb = const.tile([128, N_TILES], f32)
        nc.vector.tensor_scalar(
            out=o_sb[:], in0=p_sb[:], scalar1=cexp[:], scalar2=ginv[:],
            op0=mult, op1=mult,
        )
        nc.sync.dma_start(out_ap, o_sb[:])
        gate_out = const.tile([128, 1], f32)
        nc.vector.tensor_reduce(
            gate_out[:], o_sb[:], axis=X, op=mybir.AluOpType.max
        )
        return gate_out[:]


def _build_bass():
    import concourse.bacc as bacc
    import concourse.mybir as mybir
    import concourse.tile as tile

    f32 = mybir.dt.float32
    nc = bacc.Bacc(
        "TRN2", target_bir_lowering=False, debug=False, num_devices=N_CORES
    )
    enc_in = nc.dram_tensor("enc", [S_SHARD, H], f32, kind="ExternalInput")
    w_in = nc.dram_tensor("w", [H, V_SHARD], f32, kind="ExternalInput")
    h_in = nc.dram_tensor("h", [128, KT], f32, kind="ExternalInput")
    out = nc.dram_tensor("attn", [128, N_TILES], f32, kind="ExternalOutput")

    with tile.TileContext(nc) as tc:
        emit(tc, out.ap(), enc_in.ap(), w_in.ap(), h_in.ap())

    nc.compile()
    return nc


_NC_CACHE = None


def make_in_maps(hidden, encoder_outputs, W):
    h = np.asarray(hidden, dtype=np.float32).reshape(H)
    enc = np.asarray(encoder_outputs, dtype=np.float32).reshape(S, H)
    W = np.asarray(W, dtype=np.float32)
    h_tile = np.ascontiguousarray(h.reshape(KT, 128).T)
    in_maps = []
    for c in range(N_CORES):
        in_maps.append(
            {
                "enc": np.ascontiguousarray(enc[c * S_SHARD : (c + 1) * S_SHARD]),
                "w": np.ascontiguousarray(W[:, c * V_SHARD : (c + 1) * V_SHARD]),
                "h": h_tile,
            }
        )
    return in_maps


def kernel(hidden, encoder_outputs, W, b):
    from concourse import bass_utils

    global _NC_CACHE
    if _NC_CACHE is None:
        _NC_CACHE = _build_bass()
    nc = _NC_CACHE

    in_maps = make_in_maps(hidden, encoder_outputs, W)
    res = bass_utils.run_bass_kernel_spmd(
        nc, in_maps, core_ids=list(range(N_CORES))
    )
    shards = [r["attn"].T.reshape(S_SHARD) for r in res.results]
    return np.concatenate(shards).reshape(1, 1, S).astype(np.float32)

